# revision 14
# baseline (speedup 1.0000x reference)
"""Trainium2 Bass kernel for DeepSelfAttention (N=8192, D=1024) on 8 NeuronCores.

Strategy (row-parallel attention), v5:
  - Shard the N=8192 rows of x across 8 cores (1024 rows each); replicate
    weights, except W1/2/3 whose transposes are distributed: each core gets a
    128-row slice as input, transposes it (3us), and the fp16 W^T tiles are
    AllGathered + unpacked while attention runs.
  - K/V are AllGathered in three chunks of [256, 256, 512] keys, shipped as
    soon as each is projected; Wq^T + the Q projection fill the first
    AllGather's latency.
  - All transposes run in fp16 (cast on load via the Vector engine): 1 PE
    cycle/row instead of fp32's 2.
  - Flash-style one-pass attention (exp without max-subtraction; scores
    provably in [-3,3]).  Softmax denominator accumulated on the Vector
    engine into rs_acc[128, NS], reduced cross-partition with 2 fp32 matmuls.
  - The softmax normalize (with the V bias folded in, since softmax rows sum
    to 1) is fused into the last attention block per query-half, so the MLP
    starts with no PE gap (avoids a HAM re-throttle).
All matmul operands fp16 (full PE rate) with fp32 PSUM accumulation.
"""

import os

import numpy as np

import concourse.mybir as mybir
import concourse.tile as tile
from concourse import bacc
from concourse import bass_utils
from concourse.masks import make_identity

P = 128
D = 1024
N = 8192
NCORES = 8
NS = N // NCORES          # 1024 rows per core
DT = D // P               # 8 feature tiles
QGS = 256
KB = 8                    # k blocks (one per source core)
CHUNK_KT = (2, 2, 4)      # k-tiles per chunk: 256 + 256 + 512 keys
WSZ = 3 * DT * P * P      # per-core W1/2/3 transposed-share elements
F16 = mybir.dt.float16
F32 = mybir.dt.float32
AF = mybir.ActivationFunctionType
ALU = mybir.AluOpType

SCALE = 1.0 / np.sqrt(np.float32(D)).astype(np.float32)  # 0.03125

_CACHE = {}


def _transpose_pe(nc, raw_pool, ptr_pool, ident16, src_ap, dst_tile):
    """src_ap: DRAM fp32 [R, C] -> dst_tile: SBUF fp16 [P, C//P, R] = src.T.
    Casts to fp16 on the Vector engine first, then fp16 TensorE transpose
    (1 cycle/row) + Vector PSUM->SBUF copy."""
    R, C = src_ap.shape
    for i in range(R // P):
        r = raw_pool.tile([P, C], F32, tag="raw")
        nc.sync.dma_start(r[:], src_ap[i * P:(i + 1) * P, :])
        rh = raw_pool.tile([P, C], F16, tag="rawh")
        nc.scalar.activation(rh[:], r[:], AF.Copy)
        for j in range(C // P):
            pst = ptr_pool.tile([P, P], F16, tag="ptr")
            nc.tensor.transpose(pst[:], rh[:, j * P:(j + 1) * P], ident16[:])
            nc.vector.tensor_copy(dst_tile[:, j, i * P:(i + 1) * P], pst[:])


def _build():
    nc = bacc.Bacc("TRN2", target_bir_lowering=False, debug=False,
                   num_devices=NCORES)
    xs = nc.dram_tensor("xs", [NS, D], F32, kind="ExternalInput").ap()
    W = {}
    for w in ("wq", "wk", "wv"):
        W[w] = nc.dram_tensor(w, [D, D], F32, kind="ExternalInput").ap()
    ws_in = {w: nc.dram_tensor(f"{w}s", [P, D], F32, kind="ExternalInput").ap()
             for w in ("w1", "w2", "w3")}
    B = {}
    for b in ("bq", "bk", "bv", "b1", "b2", "b3"):
        B[b] = nc.dram_tensor(b, [D], F32, kind="ExternalInput").ap()
    fw = nc.dram_tensor("fw", [D], F32, kind="ExternalInput").ap()
    out = nc.dram_tensor("out", [1, NS], F32, kind="ExternalOutput").ap()

    ck0 = [sum(CHUNK_KT[:c]) for c in range(len(CHUNK_KT))]  # first kt of chunk

    with tile.TileContext(nc) as tc:
        with (
            tc.tile_pool(name="persist", bufs=1) as pers,
            tc.tile_pool(name="dram", bufs=1, space="DRAM") as dram,
        ):
            # ---- persistent SBUF tiles ----
            qt = pers.tile([P, DT, NS], F16, tag="qt")          # Q^T
            bsb = {b: pers.tile([P, DT], F32, tag=f"{b}sb", name=f"{b}sb")
                   for b in B}
            fwh = pers.tile([P, DT], F16, tag="fwh")
            ones_row = pers.tile([1, P], F32, tag="ones_row")
            ones_col = pers.tile([P, 1], F32, tag="ones_col")
            ident16 = pers.tile([P, P], F16, tag="ident16")
            rs_acc = pers.tile([P, NS], F32, tag="rs_acc")      # exp sums
            rs = pers.tile([1, NS], F32, tag="rs")              # softmax denom

            # ---- DRAM scratch: collective buffers ----
            kv_d, kvag = [], []
            for c, kt_n in enumerate(CHUNK_KT):
                sz = 2 * D * (P * kt_n)
                kv_d.append(dram.tile([sz], F16, name=f"kv_d{c}"))
                kvag.append(dram.tile([NCORES * sz], F16, name=f"kvag{c}",
                                      addr_space="Shared"))
            wd = dram.tile([WSZ], F16, name="wd")
            wag = dram.tile([NCORES * WSZ], F16, name="wag",
                            addr_space="Shared")

            # ---- constants ----
            for b in B:
                nc.sync.dma_start(bsb[b][:], B[b].rearrange("(t p) -> p t", p=P))
            fwf = pers.tile([P, DT], F32, tag="fwf")
            nc.sync.dma_start(fwf[:], fw.rearrange("(t p) -> p t", p=P))
            nc.vector.tensor_copy(fwh[:], fwf[:])
            nc.gpsimd.memset(ones_row[:], 1.0)
            nc.gpsimd.memset(ones_col[:], 1.0)
            nc.gpsimd.memset(rs_acc[:], 0.0)
            make_identity(nc, ident16[:])

            # ---- early pools ----
            early = tc.alloc_tile_pool(name="early", bufs=1)
            xsT = early.tile([P, DT, NS], F16, tag="xsT")
            wT = {}
            for w in ("wk", "wv"):
                wT[w] = early.tile([P, DT, D], F16, tag=f"{w}T", name=f"{w}T")
            kts = early.tile([P, DT, 512], F16, tag="kts")      # K^T chunk
            vs = early.tile([P, 4, D], F16, tag="vs")           # V chunk
            wcon = early.tile([P, 3, DT, P], F16, tag="wcon")   # W123^T share
            early_q = tc.alloc_tile_pool(name="early_q", bufs=1)
            wT["wq"] = early_q.tile([P, DT, D], F16, tag="wqT",
                                    name="wqT")

            def k_chunk(c, pp):
                """K^T for chunk c -> kts -> kv_d[c]."""
                kn = P * CHUNK_KT[c]
                q0 = ck0[c] * P
                for dt in range(DT):
                    ps = pp.tile([P, 512], F32, tag="ppj")
                    for et in range(DT):
                        nc.tensor.matmul(
                            ps[:, 0:kn],
                            wT["wk"][:, et, dt * P:(dt + 1) * P],
                            xsT[:, et, q0:q0 + kn],
                            start=(et == 0), stop=(et == DT - 1),
                            skip_group_check=True)
                    nc.vector.tensor_tensor(
                        kts[:, dt, 0:kn], ps[:, 0:kn],
                        bsb["bk"][:, dt:dt + 1].to_broadcast([P, kn]),
                        ALU.add)
                    nc.sync.dma_start(
                        kv_d[c][dt * P * kn:(dt + 1) * P * kn].rearrange(
                            "(p k) -> p k", p=P),
                        kts[:, dt, 0:kn])

            def v_chunk(c, pp):
                """V rows for chunk c -> vs -> kv_d[c]; then AllGather c."""
                kt_n = CHUNK_KT[c]
                kn = P * kt_n
                for kt in range(kt_n):
                    kta = ck0[c] + kt
                    pss = [pp.tile([P, 512], F32, tag="ppj",
                                   name=f"vps{_h}") for _h in range(2)]
                    for et in range(DT):
                        for dh in range(2):
                            nc.tensor.matmul(
                                pss[dh][:],
                                xsT[:, et, kta * P:(kta + 1) * P],
                                wT["wv"][:, et, dh * 512:(dh + 1) * 512],
                                start=(et == 0), stop=(et == DT - 1),
                                skip_group_check=True)
                    for dh in range(2):
                        nc.vector.tensor_copy(
                            vs[:, kt, dh * 512:(dh + 1) * 512], pss[dh][:])
                    o = D * kn + kt * P * D
                    nc.sync.dma_start(
                        kv_d[c][o:o + P * D].rearrange("(p d) -> p d", p=P),
                        vs[:, kt, :])
                nc.gpsimd.collective_compute(
                    "AllGather", ALU.bypass,
                    replica_groups=[list(range(NCORES))],
                    ins=[kv_d[c].opt()], outs=[kvag[c].opt()])

            with (
                tc.tile_pool(name="raw", bufs=3) as raw,
                tc.tile_pool(name="ppj", bufs=4, space="PSUM") as ppj,
            ):
                # distributed W1/2/3 transposes first: tiny (3us), and the
                # W^T AllGather completes before attention starts, so the
                # unpack DMAs below never hold up the DMA queues
                for li, w in enumerate(("w1", "w2", "w3")):
                    _transpose_pe(nc, raw, ppj, ident16, ws_in[w],
                                  wcon[:, li])
                nc.sync.dma_start(
                    wd.rearrange("(l i p e) -> p l i e", l=3, i=DT, p=P, e=P),
                    wcon[:])
                nc.gpsimd.collective_compute(
                    "AllGather", ALU.bypass,
                    replica_groups=[list(range(NCORES))],
                    ins=[wd.opt()], outs=[wag.opt()])
                # transposes gating the first K/V AllGather
                _transpose_pe(nc, raw, ppj, ident16, xs, xsT)
                for w in ("wk", "wv"):
                    _transpose_pe(nc, raw, ppj, ident16, W[w], wT[w])
                k_chunk(0, ppj)
                v_chunk(0, ppj)
                # fill AllGather-0 latency: Wq^T + Q^T projection
                _transpose_pe(nc, raw, ppj, ident16, W["wq"], wT["wq"])
                for dt in range(DT):
                    pss = [ppj.tile([P, 512], F32, tag="ppj",
                                    name=f"qps{_h}") for _h in range(2)]
                    for et in range(DT):
                        for h in range(2):
                            nc.tensor.matmul(
                                pss[h][:],
                                wT["wq"][:, et, dt * P:(dt + 1) * P],
                                xsT[:, et, h * 512:(h + 1) * 512],
                                start=(et == 0), stop=(et == DT - 1),
                                skip_group_check=True)
                    for h in range(2):
                        nc.vector.tensor_tensor(
                            qt[:, dt, h * 512:(h + 1) * 512], pss[h][:],
                            bsb["bq"][:, dt:dt + 1].to_broadcast([P, 512]),
                            ALU.add)
                k_chunk(1, ppj)
                v_chunk(1, ppj)
                k_chunk(2, ppj)
                v_chunk(2, ppj)
            early_q.release()
            early.release()

            # ---- long-lived attention outputs + MLP weights ----
            pacc = tc.alloc_tile_pool(name="pacc", bufs=1)
            attacc = pacc.tile([P, DT, NS], F32, tag="attacc")
            attn_h = pacc.tile([P, DT, NS], F16, tag="attn_h")
            recip = pacc.tile([1, NS], F32, tag="recip")
            wmlp = tc.alloc_tile_pool(name="wmlp", bufs=1)
            for l in range(3):
                wn = f"w{l + 1}"
                wT[wn] = wmlp.tile([P, DT, DT, P], F16,
                                   tag=f"{wn}T", name=f"{wn}T")
                for cc in range(NCORES):
                    o = cc * WSZ + l * DT * P * P
                    nc.sync.dma_start(
                        wT[wn][:, :, cc, :],
                        wag[o:o + DT * P * P].rearrange(
                            "(i p e) -> p i e", i=DT, p=P, e=P))

            # ---- attention over chunks x 8 blocks ----
            with (
                tc.tile_pool(name="kv", bufs=3) as kv,
                tc.tile_pool(name="ex", bufs=10) as exp_pool,
                tc.tile_pool(name="psc", bufs=3, space="PSUM") as psc,
                tc.tile_pool(name="pat", bufs=4, space="PSUM") as pat,
                tc.tile_pool(name="prs", bufs=1, space="PSUM") as prs,
            ):
                rbs = {}

                def norm_part(h, hq):
                    """Fold softmax denom + V bias into attn_h for query
                    quarter (h, hq); emitted inside the last attention block
                    so the DVE work overlaps the remaining PE work."""
                    if hq == 0:
                        qsl = slice(h * 512, (h + 1) * 512)
                        pr = prs.tile([1, 512], F32, tag="prs", name="pr")
                        nc.tensor.matmul(pr[:], ones_col[:], rs_acc[:, qsl])
                        nc.vector.tensor_copy(rs[0:1, qsl], pr[:])
                        nc.vector.reciprocal(recip[0:1, qsl], rs[0:1, qsl])
                        rb = psc.tile([P, 512], F32, tag="psc", name="rb")
                        nc.tensor.matmul(rb[:], ones_row[:], recip[0:1, qsl])
                        rbs[h] = rb
                    qsl = slice(h * 512 + hq * QGS, h * 512 + (hq + 1) * QGS)
                    rbq = rbs[h][:, hq * QGS:(hq + 1) * QGS]
                    for dt in range(DT):
                        nc.vector.tensor_tensor(
                            attn_h[:, dt, qsl], attacc[:, dt, qsl], rbq,
                            ALU.mult)
                        nc.vector.tensor_tensor(
                            attn_h[:, dt, qsl], attn_h[:, dt, qsl],
                            bsb["bv"][:, dt:dt + 1].to_broadcast([P, QGS]),
                            ALU.add)

                for c, kt_n in enumerate(CHUNK_KT):
                    kn = P * kt_n
                    base = kvag[c]
                    sz = 2 * D * kn
                    for kb in range(KB):
                        off = kb * sz
                        ktb = kv.tile([P, DT, 512], F16, tag="ktb")
                        vb = kv.tile([P, 4, D], F16, tag="vb")
                        nc.sync.dma_start(
                            ktb[:, :, 0:kn],
                            base[off:off + D * kn].rearrange(
                                "(t p k) -> p t k", p=P, k=kn))
                        nc.sync.dma_start(
                            vb[:, 0:kt_n, :],
                            base[off + D * kn:off + sz].rearrange(
                                "(t p d) -> p t d", p=P, d=D))
                        first_blk = c == 0 and kb == 0
                        last_blk = (c == len(CHUNK_KT) - 1 and kb == KB - 1)
                        exs = [[], []]
                        for kt in range(kt_n):
                            scs = [psc.tile([P, 512], F32, tag="psc",
                                            name=f"sc{_h}")
                                   for _h in range(2)]
                            for dt in range(DT):
                                for qp in range(2):
                                    nc.tensor.matmul(
                                        scs[qp][:],
                                        ktb[:, dt, kt * P:(kt + 1) * P],
                                        qt[:, dt, qp * 512:(qp + 1) * 512],
                                        start=(dt == 0), stop=(dt == DT - 1),
                                        skip_group_check=True)
                            for qp in range(2):
                                ex = exp_pool.tile([P, 512], F16, tag="ex",
                                                   name=f"ex{kt}_{qp}")
                                nc.scalar.activation(ex[:], scs[qp][:], AF.Exp,
                                                     scale=float(SCALE))
                                nc.vector.tensor_tensor(
                                    rs_acc[:, qp * 512:(qp + 1) * 512], ex[:],
                                    rs_acc[:, qp * 512:(qp + 1) * 512],
                                    ALU.add)
                                exs[qp].append(ex)
                        # A@V, one PSUM-bank accumulation group at a time
                        for qp in range(2):
                            for hq in range(2):
                                qsl = slice(qp * 512 + hq * QGS,
                                            qp * 512 + (hq + 1) * QGS)
                                att_ps = [pat.tile([P, 2, QGS], F32, tag="pat",
                                                   name=f"att_ps{_j}")
                                          for _j in range(4)]
                                for dt in range(DT):
                                    for kt in range(kt_n):
                                        nc.tensor.matmul(
                                            att_ps[dt // 2][:, dt % 2, :],
                                            vb[:, kt, dt * P:(dt + 1) * P],
                                            exs[qp][kt][:,
                                                        hq * QGS:(hq + 1) * QGS],
                                            start=(kt == 0),
                                            stop=(kt == kt_n - 1),
                                            skip_group_check=True)
                                for j in range(4):
                                    dsl = (slice(None), slice(2 * j, 2 * j + 2),
                                           qsl)
                                    if first_blk:
                                        nc.vector.tensor_copy(attacc[dsl],
                                                              att_ps[j][:])
                                    else:
                                        nc.vector.tensor_tensor(
                                            attacc[dsl], att_ps[j][:],
                                            attacc[dsl], ALU.add)
                                if last_blk:
                                    norm_part(qp, hq)

            # ---- MLP + final, per column-half ----
            with (
                tc.tile_pool(name="acts", bufs=2) as acts,
                tc.tile_pool(name="pml", bufs=4, space="PSUM") as pml,
            ):
                out_sb = acts.tile([1, NS], F32, tag="out_sb")
                cur = attn_h
                for li, (wname, bname) in enumerate(
                        (("w1", "b1"), ("w2", "b2"), ("w3", "b3"))):
                    nxt = acts.tile([P, DT, NS], F16, tag="y", name=f"y{li}")
                    for ft in range(DT):
                        pss = [pml.tile([P, 512], F32, tag="pml",
                                        name=f"mps{_h}")
                               for _h in range(2)]
                        for dt in range(DT):
                            for h in range(2):
                                nc.tensor.matmul(
                                    pss[h][:],
                                    wT[wname][:, dt, ft, :],
                                    cur[:, dt, h * 512:(h + 1) * 512],
                                    start=(dt == 0), stop=(dt == DT - 1),
                                    skip_group_check=True)
                        for h in range(2):
                            nc.scalar.activation(
                                nxt[:, ft, h * 512:(h + 1) * 512], pss[h][:],
                                AF.Relu, bias=bsb[bname][:, ft:ft + 1])
                    cur = nxt
                for h in range(2):
                    ps = pml.tile([1, 512], F32, tag="pfin")
                    for ft in range(DT):
                        nc.tensor.matmul(
                            ps[:], fwh[:, ft:ft + 1],
                            cur[:, ft, h * 512:(h + 1) * 512],
                            start=(ft == 0), stop=(ft == DT - 1))
                    nc.vector.tensor_copy(out_sb[0:1, h * 512:(h + 1) * 512],
                                          ps[:])
                nc.sync.dma_start(out[:], out_sb[:])
            wmlp.release()
            pacc.release()

    nc.compile()
    return nc


def _get_nc():
    if "nc" not in _CACHE:
        _CACHE["nc"] = _build()
    return _CACHE["nc"]


def _in_maps(inputs):
    x = np.ascontiguousarray(np.asarray(inputs["x"], dtype=np.float32))
    names = {"wq": "Wq", "wk": "Wk", "wv": "Wv",
             "bq": "bq", "bk": "bk", "bv": "bv", "b1": "b1",
             "b2": "b2", "b3": "b3"}
    shared = {k: np.ascontiguousarray(np.asarray(inputs[v], dtype=np.float32))
              for k, v in names.items()}
    shared["fw"] = np.ascontiguousarray(
        np.asarray(inputs["final_weight"], dtype=np.float32).reshape(D))
    w123 = [np.asarray(inputs[w], dtype=np.float32)
            for w in ("W1", "W2", "W3")]
    in_maps = []
    for c in range(NCORES):
        m = dict(shared)
        m["xs"] = np.ascontiguousarray(x[c * NS:(c + 1) * NS, :])
        for l, wfull in enumerate(w123):
            m[f"w{l + 1}s"] = np.ascontiguousarray(
                wfull[c * P:(c + 1) * P, :])
        in_maps.append(m)
    return in_maps


def kernel(**inputs):
    nc = _get_nc()
    res = bass_utils.run_bass_kernel_spmd(
        nc, _in_maps(inputs), core_ids=list(range(NCORES)))
    return np.concatenate(
        [res.results[c]["out"].reshape(NS) for c in range(NCORES)])


# revision 15
# speedup vs baseline: 1.0049x; 1.0049x over previous
"""Trainium2 Bass kernel for DeepSelfAttention (N=8192, D=1024) on 8 NeuronCores.

Strategy (row-parallel attention), v5:
  - Shard the N=8192 rows of x across 8 cores (1024 rows each); replicate
    weights, except W1/2/3 whose transposes are distributed: each core gets a
    128-row slice as input, transposes it (3us), and the fp16 W^T tiles are
    AllGathered + unpacked while attention runs.
  - K/V are AllGathered in three chunks of [256, 256, 512] keys, shipped as
    soon as each is projected; Wq^T + the Q projection fill the first
    AllGather's latency.
  - All transposes run in fp16 (cast on load via the Vector engine): 1 PE
    cycle/row instead of fp32's 2.
  - Flash-style one-pass attention (exp without max-subtraction; scores
    provably in [-3,3]).  Softmax denominator accumulated on the Vector
    engine into rs_acc[128, NS], reduced cross-partition with 2 fp32 matmuls.
  - The softmax normalize (with the V bias folded in, since softmax rows sum
    to 1) is fused into the last attention block per query-half, so the MLP
    starts with no PE gap (avoids a HAM re-throttle).
All matmul operands fp16 (full PE rate) with fp32 PSUM accumulation.
"""

import os

import numpy as np

import concourse.mybir as mybir
import concourse.tile as tile
from concourse import bacc
from concourse import bass_utils
from concourse.masks import make_identity

P = 128
D = 1024
N = 8192
NCORES = 8
NS = N // NCORES          # 1024 rows per core
DT = D // P               # 8 feature tiles
QGS = 256
KB = 8                    # k blocks (one per source core)
CHUNK_KT = (2, 2, 4)      # k-tiles per chunk: 256 + 256 + 512 keys
WSZ = 3 * DT * P * P      # per-core W1/2/3 transposed-share elements
F16 = mybir.dt.float16
F32 = mybir.dt.float32
AF = mybir.ActivationFunctionType
ALU = mybir.AluOpType

SCALE = 1.0 / np.sqrt(np.float32(D)).astype(np.float32)  # 0.03125

_CACHE = {}


def _transpose_pe(nc, raw_pool, ptr_pool, ident16, src_ap, dst_tile):
    """src_ap: DRAM fp32 [R, C] -> dst_tile: SBUF fp16 [P, C//P, R] = src.T.
    Casts to fp16 on the Vector engine first, then fp16 TensorE transpose
    (1 cycle/row) + Vector PSUM->SBUF copy."""
    R, C = src_ap.shape
    for i in range(R // P):
        r = raw_pool.tile([P, C], F32, tag="raw")
        nc.sync.dma_start(r[:], src_ap[i * P:(i + 1) * P, :])
        rh = raw_pool.tile([P, C], F16, tag="rawh")
        nc.scalar.activation(rh[:], r[:], AF.Copy)
        for j in range(C // P):
            pst = ptr_pool.tile([P, P], F16, tag="ptr")
            nc.tensor.transpose(pst[:], rh[:, j * P:(j + 1) * P], ident16[:])
            nc.vector.tensor_copy(dst_tile[:, j, i * P:(i + 1) * P], pst[:])


def _build():
    nc = bacc.Bacc("TRN2", target_bir_lowering=False, debug=False,
                   num_devices=NCORES)
    xs = nc.dram_tensor("xs", [NS, D], F32, kind="ExternalInput").ap()
    W = {}
    for w in ("wq", "wk", "wv"):
        W[w] = nc.dram_tensor(w, [D, D], F32, kind="ExternalInput").ap()
    ws_in = {w: nc.dram_tensor(f"{w}s", [P, D], F32, kind="ExternalInput").ap()
             for w in ("w1", "w2", "w3")}
    B = {}
    for b in ("bq", "bk", "bv", "b1", "b2", "b3"):
        B[b] = nc.dram_tensor(b, [D], F32, kind="ExternalInput").ap()
    fw = nc.dram_tensor("fw", [D], F32, kind="ExternalInput").ap()
    out = nc.dram_tensor("out", [1, NS], F32, kind="ExternalOutput").ap()

    ck0 = [sum(CHUNK_KT[:c]) for c in range(len(CHUNK_KT))]  # first kt of chunk

    with tile.TileContext(nc) as tc:
        with (
            tc.tile_pool(name="persist", bufs=1) as pers,
            tc.tile_pool(name="dram", bufs=1, space="DRAM") as dram,
        ):
            # ---- persistent SBUF tiles ----
            qt = pers.tile([P, DT, NS], F16, tag="qt")          # Q^T
            bsb = {b: pers.tile([P, DT], F32, tag=f"{b}sb", name=f"{b}sb")
                   for b in B}
            fwh = pers.tile([P, DT], F16, tag="fwh")
            ones_row = pers.tile([1, P], F32, tag="ones_row")
            ones_col = pers.tile([P, 1], F32, tag="ones_col")
            ident16 = pers.tile([P, P], F16, tag="ident16")
            rs_acc = pers.tile([P, NS], F32, tag="rs_acc")      # exp sums
            rs = pers.tile([1, NS], F32, tag="rs")              # softmax denom

            # ---- DRAM scratch: collective buffers ----
            kv_d, kvag = [], []
            for c, kt_n in enumerate(CHUNK_KT):
                sz = 2 * D * (P * kt_n)
                kv_d.append(dram.tile([sz], F16, name=f"kv_d{c}"))
                kvag.append(dram.tile([NCORES * sz], F16, name=f"kvag{c}",
                                      addr_space="Shared"))
            wd = dram.tile([WSZ], F16, name="wd")
            wag = dram.tile([NCORES * WSZ], F16, name="wag",
                            addr_space="Shared")

            # ---- constants ----
            for b in B:
                nc.sync.dma_start(bsb[b][:], B[b].rearrange("(t p) -> p t", p=P))
            fwf = pers.tile([P, DT], F32, tag="fwf")
            nc.sync.dma_start(fwf[:], fw.rearrange("(t p) -> p t", p=P))
            nc.vector.tensor_copy(fwh[:], fwf[:])
            nc.gpsimd.memset(ones_row[:], 1.0)
            nc.gpsimd.memset(ones_col[:], 1.0)
            nc.gpsimd.memset(rs_acc[:], 0.0)
            make_identity(nc, ident16[:])

            # ---- early pools ----
            early = tc.alloc_tile_pool(name="early", bufs=1)
            xsT = early.tile([P, DT, NS], F16, tag="xsT")
            wT = {}
            for w in ("wk", "wv"):
                wT[w] = early.tile([P, DT, D], F16, tag=f"{w}T", name=f"{w}T")
            kts = early.tile([P, DT, 512], F16, tag="kts")      # K^T chunk
            vs = early.tile([P, 4, D], F16, tag="vs")           # V chunk
            wcon = early.tile([P, 3, DT, P], F16, tag="wcon")   # W123^T share
            early_q = tc.alloc_tile_pool(name="early_q", bufs=1)
            wT["wq"] = early_q.tile([P, DT, D], F16, tag="wqT",
                                    name="wqT")

            def k_chunk(c, pp):
                """K^T for chunk c -> kts -> kv_d[c]."""
                kn = P * CHUNK_KT[c]
                q0 = ck0[c] * P
                for dt in range(DT):
                    ps = pp.tile([P, 512], F32, tag="ppj")
                    for et in range(DT):
                        nc.tensor.matmul(
                            ps[:, 0:kn],
                            wT["wk"][:, et, dt * P:(dt + 1) * P],
                            xsT[:, et, q0:q0 + kn],
                            start=(et == 0), stop=(et == DT - 1),
                            skip_group_check=True)
                    nc.vector.tensor_tensor(
                        kts[:, dt, 0:kn], ps[:, 0:kn],
                        bsb["bk"][:, dt:dt + 1].to_broadcast([P, kn]),
                        ALU.add)
                    nc.sync.dma_start(
                        kv_d[c][dt * P * kn:(dt + 1) * P * kn].rearrange(
                            "(p k) -> p k", p=P),
                        kts[:, dt, 0:kn])

            def v_chunk(c, pp):
                """V rows for chunk c -> vs -> kv_d[c]; then AllGather c."""
                kt_n = CHUNK_KT[c]
                kn = P * kt_n
                for kt in range(kt_n):
                    kta = ck0[c] + kt
                    pss = [pp.tile([P, 512], F32, tag="ppj",
                                   name=f"vps{_h}") for _h in range(2)]
                    for et in range(DT):
                        for dh in range(2):
                            nc.tensor.matmul(
                                pss[dh][:],
                                xsT[:, et, kta * P:(kta + 1) * P],
                                wT["wv"][:, et, dh * 512:(dh + 1) * 512],
                                start=(et == 0), stop=(et == DT - 1),
                                skip_group_check=True)
                    for dh in range(2):
                        nc.vector.tensor_copy(
                            vs[:, kt, dh * 512:(dh + 1) * 512], pss[dh][:])
                    o = D * kn + kt * P * D
                    nc.sync.dma_start(
                        kv_d[c][o:o + P * D].rearrange("(p d) -> p d", p=P),
                        vs[:, kt, :])
                nc.gpsimd.collective_compute(
                    "AllGather", ALU.bypass,
                    replica_groups=[list(range(NCORES))],
                    ins=[kv_d[c].opt()], outs=[kvag[c].opt()])

            with (
                tc.tile_pool(name="raw", bufs=3) as raw,
                tc.tile_pool(name="ppj", bufs=4, space="PSUM") as ppj,
            ):
                # distributed W1/2/3 transposes first: tiny (3us), and the
                # W^T AllGather completes before attention starts, so the
                # unpack DMAs below never hold up the DMA queues
                for li, w in enumerate(("w1", "w2", "w3")):
                    _transpose_pe(nc, raw, ppj, ident16, ws_in[w],
                                  wcon[:, li])
                nc.sync.dma_start(
                    wd.rearrange("(l i p e) -> p l i e", l=3, i=DT, p=P, e=P),
                    wcon[:])
                # transposes gating the first K/V AllGather
                _transpose_pe(nc, raw, ppj, ident16, xs, xsT)
                for w in ("wk", "wv"):
                    _transpose_pe(nc, raw, ppj, ident16, W[w], wT[w])
                k_chunk(0, ppj)
                v_chunk(0, ppj)
                # W^T AllGather second: small, and done (~170us) before the
                # unpack DMAs emitted in chunk-0 block 2 reach a queue head
                nc.gpsimd.collective_compute(
                    "AllGather", ALU.bypass,
                    replica_groups=[list(range(NCORES))],
                    ins=[wd.opt()], outs=[wag.opt()])
                # fill AllGather-0 latency: Wq^T + Q^T projection
                _transpose_pe(nc, raw, ppj, ident16, W["wq"], wT["wq"])
                for dt in range(DT):
                    pss = [ppj.tile([P, 512], F32, tag="ppj",
                                    name=f"qps{_h}") for _h in range(2)]
                    for et in range(DT):
                        for h in range(2):
                            nc.tensor.matmul(
                                pss[h][:],
                                wT["wq"][:, et, dt * P:(dt + 1) * P],
                                xsT[:, et, h * 512:(h + 1) * 512],
                                start=(et == 0), stop=(et == DT - 1),
                                skip_group_check=True)
                    for h in range(2):
                        nc.vector.tensor_tensor(
                            qt[:, dt, h * 512:(h + 1) * 512], pss[h][:],
                            bsb["bq"][:, dt:dt + 1].to_broadcast([P, 512]),
                            ALU.add)
                k_chunk(1, ppj)
                v_chunk(1, ppj)
                k_chunk(2, ppj)
                v_chunk(2, ppj)
            early_q.release()
            early.release()

            # ---- long-lived attention outputs + MLP weights ----
            pacc = tc.alloc_tile_pool(name="pacc", bufs=1)
            attacc = pacc.tile([P, DT, NS], F32, tag="attacc")
            attn_h = pacc.tile([P, DT, NS], F16, tag="attn_h")
            recip = pacc.tile([1, NS], F32, tag="recip")
            wmlp = tc.alloc_tile_pool(name="wmlp", bufs=1)
            for l in range(3):
                wn = f"w{l + 1}"
                wT[wn] = wmlp.tile([P, DT, DT, P], F16,
                                   tag=f"{wn}T", name=f"{wn}T")

            # ---- attention over chunks x 8 blocks ----
            with (
                tc.tile_pool(name="kv", bufs=3) as kv,
                tc.tile_pool(name="ex", bufs=10) as exp_pool,
                tc.tile_pool(name="psc", bufs=3, space="PSUM") as psc,
                tc.tile_pool(name="pat", bufs=4, space="PSUM") as pat,
                tc.tile_pool(name="prs", bufs=1, space="PSUM") as prs,
            ):
                rbs = {}

                def norm_part(h, hq):
                    """Fold softmax denom + V bias into attn_h for query
                    quarter (h, hq); emitted inside the last attention block
                    so the DVE work overlaps the remaining PE work."""
                    if hq == 0:
                        qsl = slice(h * 512, (h + 1) * 512)
                        pr = prs.tile([1, 512], F32, tag="prs", name="pr")
                        nc.tensor.matmul(pr[:], ones_col[:], rs_acc[:, qsl])
                        nc.vector.tensor_copy(rs[0:1, qsl], pr[:])
                        nc.vector.reciprocal(recip[0:1, qsl], rs[0:1, qsl])
                        rb = psc.tile([P, 512], F32, tag="psc", name="rb")
                        nc.tensor.matmul(rb[:], ones_row[:], recip[0:1, qsl])
                        rbs[h] = rb
                    qsl = slice(h * 512 + hq * QGS, h * 512 + (hq + 1) * QGS)
                    rbq = rbs[h][:, hq * QGS:(hq + 1) * QGS]
                    for dt in range(DT):
                        nc.vector.tensor_tensor(
                            attn_h[:, dt, qsl], attacc[:, dt, qsl], rbq,
                            ALU.mult)
                        nc.vector.tensor_tensor(
                            attn_h[:, dt, qsl], attn_h[:, dt, qsl],
                            bsb["bv"][:, dt:dt + 1].to_broadcast([P, QGS]),
                            ALU.add)

                for c, kt_n in enumerate(CHUNK_KT):
                    kn = P * kt_n
                    base = kvag[c]
                    sz = 2 * D * kn
                    for kb in range(KB):
                        off = kb * sz
                        ktb = kv.tile([P, DT, 512], F16, tag="ktb")
                        vb = kv.tile([P, 4, D], F16, tag="vb")
                        nc.sync.dma_start(
                            ktb[:, :, 0:kn],
                            base[off:off + D * kn].rearrange(
                                "(t p k) -> p t k", p=P, k=kn))
                        nc.sync.dma_start(
                            vb[:, 0:kt_n, :],
                            base[off + D * kn:off + sz].rearrange(
                                "(t p d) -> p t d", p=P, d=D))
                        first_blk = c == 0 and kb == 0
                        last_blk = (c == len(CHUNK_KT) - 1 and kb == KB - 1)
                        if c == 0 and kb == 2:
                            for l in range(3):
                                wn = f"w{l + 1}"
                                for cc in range(NCORES):
                                    o = cc * WSZ + l * DT * P * P
                                    nc.sync.dma_start(
                                        wT[wn][:, :, cc, :],
                                        wag[o:o + DT * P * P].rearrange(
                                            "(i p e) -> p i e", i=DT, p=P,
                                            e=P))
                        exs = [[], []]
                        for kt in range(kt_n):
                            scs = [psc.tile([P, 512], F32, tag="psc",
                                            name=f"sc{_h}")
                                   for _h in range(2)]
                            for dt in range(DT):
                                for qp in range(2):
                                    nc.tensor.matmul(
                                        scs[qp][:],
                                        ktb[:, dt, kt * P:(kt + 1) * P],
                                        qt[:, dt, qp * 512:(qp + 1) * 512],
                                        start=(dt == 0), stop=(dt == DT - 1),
                                        skip_group_check=True)
                            for qp in range(2):
                                ex = exp_pool.tile([P, 512], F16, tag="ex",
                                                   name=f"ex{kt}_{qp}")
                                nc.scalar.activation(ex[:], scs[qp][:], AF.Exp,
                                                     scale=float(SCALE))
                                nc.vector.tensor_tensor(
                                    rs_acc[:, qp * 512:(qp + 1) * 512], ex[:],
                                    rs_acc[:, qp * 512:(qp + 1) * 512],
                                    ALU.add)
                                exs[qp].append(ex)
                        # A@V, one PSUM-bank accumulation group at a time
                        for qp in range(2):
                            for hq in range(2):
                                qsl = slice(qp * 512 + hq * QGS,
                                            qp * 512 + (hq + 1) * QGS)
                                att_ps = [pat.tile([P, 2, QGS], F32, tag="pat",
                                                   name=f"att_ps{_j}")
                                          for _j in range(4)]
                                for dt in range(DT):
                                    for kt in range(kt_n):
                                        nc.tensor.matmul(
                                            att_ps[dt // 2][:, dt % 2, :],
                                            vb[:, kt, dt * P:(dt + 1) * P],
                                            exs[qp][kt][:,
                                                        hq * QGS:(hq + 1) * QGS],
                                            start=(kt == 0),
                                            stop=(kt == kt_n - 1),
                                            skip_group_check=True)
                                for j in range(4):
                                    dsl = (slice(None), slice(2 * j, 2 * j + 2),
                                           qsl)
                                    if first_blk:
                                        nc.vector.tensor_copy(attacc[dsl],
                                                              att_ps[j][:])
                                    else:
                                        nc.vector.tensor_tensor(
                                            attacc[dsl], att_ps[j][:],
                                            attacc[dsl], ALU.add)
                                if last_blk:
                                    norm_part(qp, hq)

            # ---- MLP + final, per column-half ----
            with (
                tc.tile_pool(name="acts", bufs=2) as acts,
                tc.tile_pool(name="pml", bufs=4, space="PSUM") as pml,
            ):
                out_sb = acts.tile([1, NS], F32, tag="out_sb")
                cur = attn_h
                for li, (wname, bname) in enumerate(
                        (("w1", "b1"), ("w2", "b2"), ("w3", "b3"))):
                    nxt = acts.tile([P, DT, NS], F16, tag="y", name=f"y{li}")
                    for ft in range(DT):
                        pss = [pml.tile([P, 512], F32, tag="pml",
                                        name=f"mps{_h}")
                               for _h in range(2)]
                        for dt in range(DT):
                            for h in range(2):
                                nc.tensor.matmul(
                                    pss[h][:],
                                    wT[wname][:, dt, ft, :],
                                    cur[:, dt, h * 512:(h + 1) * 512],
                                    start=(dt == 0), stop=(dt == DT - 1),
                                    skip_group_check=True)
                        for h in range(2):
                            nc.scalar.activation(
                                nxt[:, ft, h * 512:(h + 1) * 512], pss[h][:],
                                AF.Relu, bias=bsb[bname][:, ft:ft + 1])
                    cur = nxt
                for h in range(2):
                    ps = pml.tile([1, 512], F32, tag="pfin")
                    for ft in range(DT):
                        nc.tensor.matmul(
                            ps[:], fwh[:, ft:ft + 1],
                            cur[:, ft, h * 512:(h + 1) * 512],
                            start=(ft == 0), stop=(ft == DT - 1))
                    nc.vector.tensor_copy(out_sb[0:1, h * 512:(h + 1) * 512],
                                          ps[:])
                nc.sync.dma_start(out[:], out_sb[:])
            wmlp.release()
            pacc.release()

    nc.compile()
    return nc


def _get_nc():
    if "nc" not in _CACHE:
        _CACHE["nc"] = _build()
    return _CACHE["nc"]


def _in_maps(inputs):
    x = np.ascontiguousarray(np.asarray(inputs["x"], dtype=np.float32))
    names = {"wq": "Wq", "wk": "Wk", "wv": "Wv",
             "bq": "bq", "bk": "bk", "bv": "bv", "b1": "b1",
             "b2": "b2", "b3": "b3"}
    shared = {k: np.ascontiguousarray(np.asarray(inputs[v], dtype=np.float32))
              for k, v in names.items()}
    shared["fw"] = np.ascontiguousarray(
        np.asarray(inputs["final_weight"], dtype=np.float32).reshape(D))
    w123 = [np.asarray(inputs[w], dtype=np.float32)
            for w in ("W1", "W2", "W3")]
    in_maps = []
    for c in range(NCORES):
        m = dict(shared)
        m["xs"] = np.ascontiguousarray(x[c * NS:(c + 1) * NS, :])
        for l, wfull in enumerate(w123):
            m[f"w{l + 1}s"] = np.ascontiguousarray(
                wfull[c * P:(c + 1) * P, :])
        in_maps.append(m)
    return in_maps


def kernel(**inputs):
    nc = _get_nc()
    res = bass_utils.run_bass_kernel_spmd(
        nc, _in_maps(inputs), core_ids=list(range(NCORES)))
    return np.concatenate(
        [res.results[c]["out"].reshape(NS) for c in range(NCORES)])


# revision 17
# speedup vs baseline: 1.0204x; 1.0155x over previous
"""Trainium2 Bass kernel for DeepSelfAttention (N=8192, D=1024) on 8 NeuronCores.

Strategy (row-parallel attention), v5:
  - Shard the N=8192 rows of x across 8 cores (1024 rows each); replicate
    weights, except W1/2/3 whose transposes are distributed: each core gets a
    128-row slice as input, transposes it (3us), and the fp16 W^T tiles are
    AllGathered + unpacked while attention runs.
  - K/V are AllGathered in three chunks of [256, 256, 512] keys, shipped as
    soon as each is projected; Wq^T + the Q projection fill the first
    AllGather's latency.
  - All transposes run in fp16 (cast on load via the Vector engine): 1 PE
    cycle/row instead of fp32's 2.
  - Flash-style one-pass attention (exp without max-subtraction; scores
    provably in [-3,3]).  Softmax denominator accumulated on the Vector
    engine into rs_acc[128, NS], reduced cross-partition with 2 fp32 matmuls.
  - The softmax normalize (with the V bias folded in, since softmax rows sum
    to 1) is fused into the last attention block per query-half, so the MLP
    starts with no PE gap (avoids a HAM re-throttle).
All matmul operands fp16 (full PE rate) with fp32 PSUM accumulation.
"""

import os

import numpy as np

import concourse.mybir as mybir
import concourse.tile as tile
from concourse import bacc
from concourse import bass_utils
from concourse.masks import make_identity

P = 128
D = 1024
N = 8192
NCORES = 8
NS = N // NCORES          # 1024 rows per core
DT = D // P               # 8 feature tiles
QGS = 256
KB = 8                    # k blocks (one per source core)
CHUNK_KT = (2, 2, 4)      # k-tiles per chunk: 256 + 256 + 512 keys
WSZ = 3 * DT * P * P      # per-core W1/2/3 transposed-share elements
F16 = mybir.dt.float16
F32 = mybir.dt.float32
AF = mybir.ActivationFunctionType
ALU = mybir.AluOpType

SCALE = 1.0 / np.sqrt(np.float32(D)).astype(np.float32)  # 0.03125

_CACHE = {}


def _transpose_pe(nc, raw_pool, ptr_pool, ident16, src_ap, dst_tile):
    """src_ap: DRAM fp32 [R, C] -> dst_tile: SBUF fp16 [P, C//P, R] = src.T.
    Casts to fp16 on the Vector engine first, then fp16 TensorE transpose
    (1 cycle/row) + Vector PSUM->SBUF copy."""
    R, C = src_ap.shape
    for i in range(R // P):
        r = raw_pool.tile([P, C], F32, tag="raw")
        nc.sync.dma_start(r[:], src_ap[i * P:(i + 1) * P, :])
        rh = raw_pool.tile([P, C], F16, tag="rawh")
        nc.scalar.activation(rh[:], r[:], AF.Copy)
        for j in range(C // P):
            pst = ptr_pool.tile([P, P], F16, tag="ptr")
            nc.tensor.transpose(pst[:], rh[:, j * P:(j + 1) * P], ident16[:])
            nc.vector.tensor_copy(dst_tile[:, j, i * P:(i + 1) * P], pst[:])


def _build():
    nc = bacc.Bacc("TRN2", target_bir_lowering=False, debug=False,
                   num_devices=NCORES)
    xs = nc.dram_tensor("xs", [NS, D], F32, kind="ExternalInput").ap()
    W = {}
    for w in ("wq", "wk", "wv"):
        W[w] = nc.dram_tensor(w, [D, D], F32, kind="ExternalInput").ap()
    ws_in = {w: nc.dram_tensor(f"{w}s", [P, D], F32, kind="ExternalInput").ap()
             for w in ("w1", "w2", "w3")}
    B = {}
    for b in ("bq", "bk", "bv", "b1", "b2", "b3"):
        B[b] = nc.dram_tensor(b, [D], F32, kind="ExternalInput").ap()
    fw = nc.dram_tensor("fw", [D], F32, kind="ExternalInput").ap()
    out = nc.dram_tensor("out", [1, NS], F32, kind="ExternalOutput").ap()

    ck0 = [sum(CHUNK_KT[:c]) for c in range(len(CHUNK_KT))]  # first kt of chunk

    with tile.TileContext(nc) as tc:
        with (
            tc.tile_pool(name="persist", bufs=1) as pers,
            tc.tile_pool(name="dram", bufs=1, space="DRAM") as dram,
        ):
            # ---- persistent SBUF tiles ----
            qt = pers.tile([P, DT, NS], F16, tag="qt")          # Q^T
            bsb = {b: pers.tile([P, DT], F32, tag=f"{b}sb", name=f"{b}sb")
                   for b in B}
            fwh = pers.tile([P, DT], F16, tag="fwh")
            ones_row = pers.tile([1, P], F32, tag="ones_row")
            ones_col = pers.tile([P, 1], F32, tag="ones_col")
            ident16 = pers.tile([P, P], F16, tag="ident16")
            rs_acc = pers.tile([P, NS], F32, tag="rs_acc")      # exp sums
            rs = pers.tile([1, NS], F32, tag="rs")              # softmax denom

            # ---- DRAM scratch: collective buffers ----
            kv_d, kvag = [], []
            last_c = len(CHUNK_KT) - 1
            for c, kt_n in enumerate(CHUNK_KT):
                sz = 2 * D * (P * kt_n) + (WSZ if c == last_c else 0)
                kv_d.append(dram.tile([sz], F16, name=f"kv_d{c}"))
                kvag.append(dram.tile([NCORES * sz], F16, name=f"kvag{c}",
                                      addr_space="Shared"))
            woff = 2 * D * (P * CHUNK_KT[last_c])

            # ---- constants ----
            for b in B:
                nc.sync.dma_start(bsb[b][:], B[b].rearrange("(t p) -> p t", p=P))
            fwf = pers.tile([P, DT], F32, tag="fwf")
            nc.sync.dma_start(fwf[:], fw.rearrange("(t p) -> p t", p=P))
            nc.vector.tensor_copy(fwh[:], fwf[:])
            nc.gpsimd.memset(ones_row[:], 1.0)
            nc.gpsimd.memset(ones_col[:], 1.0)
            nc.gpsimd.memset(rs_acc[:], 0.0)
            make_identity(nc, ident16[:])

            # ---- early pools ----
            early = tc.alloc_tile_pool(name="early", bufs=1)
            xsT = early.tile([P, DT, NS], F16, tag="xsT")
            wT = {}
            for w in ("wk", "wv"):
                wT[w] = early.tile([P, DT, D], F16, tag=f"{w}T", name=f"{w}T")
            kts = early.tile([P, DT, 512], F16, tag="kts")      # K^T chunk
            vs = early.tile([P, 4, D], F16, tag="vs")           # V chunk
            wcon = early.tile([P, 3, DT, P], F16, tag="wcon")   # W123^T share
            early_q = tc.alloc_tile_pool(name="early_q", bufs=1)
            wT["wq"] = early_q.tile([P, DT, D], F16, tag="wqT",
                                    name="wqT")

            def k_chunk(c, pp):
                """K^T for chunk c -> kts -> kv_d[c]."""
                kn = P * CHUNK_KT[c]
                q0 = ck0[c] * P
                for dt in range(DT):
                    ps = pp.tile([P, 512], F32, tag="ppj")
                    for et in range(DT):
                        nc.tensor.matmul(
                            ps[:, 0:kn],
                            wT["wk"][:, et, dt * P:(dt + 1) * P],
                            xsT[:, et, q0:q0 + kn],
                            start=(et == 0), stop=(et == DT - 1),
                            skip_group_check=True)
                    nc.vector.tensor_tensor(
                        kts[:, dt, 0:kn], ps[:, 0:kn],
                        bsb["bk"][:, dt:dt + 1].to_broadcast([P, kn]),
                        ALU.add)
                    nc.sync.dma_start(
                        kv_d[c][dt * P * kn:(dt + 1) * P * kn].rearrange(
                            "(p k) -> p k", p=P),
                        kts[:, dt, 0:kn])

            def v_chunk(c, pp):
                """V rows for chunk c -> vs -> kv_d[c]; then AllGather c."""
                kt_n = CHUNK_KT[c]
                kn = P * kt_n
                for kt in range(kt_n):
                    kta = ck0[c] + kt
                    pss = [pp.tile([P, 512], F32, tag="ppj",
                                   name=f"vps{_h}") for _h in range(2)]
                    for et in range(DT):
                        for dh in range(2):
                            nc.tensor.matmul(
                                pss[dh][:],
                                xsT[:, et, kta * P:(kta + 1) * P],
                                wT["wv"][:, et, dh * 512:(dh + 1) * 512],
                                start=(et == 0), stop=(et == DT - 1),
                                skip_group_check=True)
                    for dh in range(2):
                        nc.vector.tensor_copy(
                            vs[:, kt, dh * 512:(dh + 1) * 512], pss[dh][:])
                    o = D * kn + kt * P * D
                    nc.sync.dma_start(
                        kv_d[c][o:o + P * D].rearrange("(p d) -> p d", p=P),
                        vs[:, kt, :])
                nc.gpsimd.collective_compute(
                    "AllGather", ALU.bypass,
                    replica_groups=[list(range(NCORES))],
                    ins=[kv_d[c].opt()], outs=[kvag[c].opt()])

            with (
                tc.tile_pool(name="raw", bufs=3) as raw,
                tc.tile_pool(name="ppj", bufs=4, space="PSUM") as ppj,
            ):
                # distributed W1/2/3 transposes first: tiny (3us), and the
                # W^T AllGather completes before attention starts, so the
                # unpack DMAs below never hold up the DMA queues
                for li, w in enumerate(("w1", "w2", "w3")):
                    _transpose_pe(nc, raw, ppj, ident16, ws_in[w],
                                  wcon[:, li])
                nc.sync.dma_start(
                    kv_d[last_c][woff:].rearrange(
                        "(p l i e) -> p l i e", l=3, i=DT, p=P, e=P),
                    wcon[:])
                # transposes gating the first K/V AllGather
                _transpose_pe(nc, raw, ppj, ident16, xs, xsT)
                for w in ("wk", "wv"):
                    _transpose_pe(nc, raw, ppj, ident16, W[w], wT[w])
                k_chunk(0, ppj)
                v_chunk(0, ppj)
                # fill AllGather-0 latency: Wq^T + Q^T projection
                _transpose_pe(nc, raw, ppj, ident16, W["wq"], wT["wq"])
                for dt in range(DT):
                    pss = [ppj.tile([P, 512], F32, tag="ppj",
                                    name=f"qps{_h}") for _h in range(2)]
                    for et in range(DT):
                        for h in range(2):
                            nc.tensor.matmul(
                                pss[h][:],
                                wT["wq"][:, et, dt * P:(dt + 1) * P],
                                xsT[:, et, h * 512:(h + 1) * 512],
                                start=(et == 0), stop=(et == DT - 1),
                                skip_group_check=True)
                    for h in range(2):
                        nc.vector.tensor_tensor(
                            qt[:, dt, h * 512:(h + 1) * 512], pss[h][:],
                            bsb["bq"][:, dt:dt + 1].to_broadcast([P, 512]),
                            ALU.add)
                k_chunk(1, ppj)
                v_chunk(1, ppj)
                k_chunk(2, ppj)
                v_chunk(2, ppj)
            early_q.release()
            early.release()

            # ---- long-lived attention outputs + MLP weights ----
            pacc = tc.alloc_tile_pool(name="pacc", bufs=1)
            attacc = pacc.tile([P, DT, NS], F32, tag="attacc")
            attn_h = pacc.tile([P, DT, NS], F16, tag="attn_h")
            recip = pacc.tile([1, NS], F32, tag="recip")
            wmlp = tc.alloc_tile_pool(name="wmlp", bufs=1)
            wsh = [wmlp.tile([P, 3, DT, P], F16, tag=f"wsh{cc}",
                             name=f"wsh{cc}") for cc in range(NCORES)]

            # ---- attention over chunks x 8 blocks ----
            with (
                tc.tile_pool(name="kv", bufs=3) as kv,
                tc.tile_pool(name="ex", bufs=10) as exp_pool,
                tc.tile_pool(name="psc", bufs=3, space="PSUM") as psc,
                tc.tile_pool(name="pat", bufs=4, space="PSUM") as pat,
                tc.tile_pool(name="prs", bufs=1, space="PSUM") as prs,
            ):
                rbs = {}

                def norm_part(h, hq):
                    """Fold softmax denom + V bias into attn_h for query
                    quarter (h, hq); emitted inside the last attention block
                    so the DVE work overlaps the remaining PE work."""
                    if hq == 0:
                        qsl = slice(h * 512, (h + 1) * 512)
                        pr = prs.tile([1, 512], F32, tag="prs", name="pr")
                        nc.tensor.matmul(pr[:], ones_col[:], rs_acc[:, qsl])
                        nc.vector.tensor_copy(rs[0:1, qsl], pr[:])
                        nc.vector.reciprocal(recip[0:1, qsl], rs[0:1, qsl])
                        rb = psc.tile([P, 512], F32, tag="psc", name="rb")
                        nc.tensor.matmul(rb[:], ones_row[:], recip[0:1, qsl])
                        rbs[h] = rb
                    qsl = slice(h * 512 + hq * QGS, h * 512 + (hq + 1) * QGS)
                    rbq = rbs[h][:, hq * QGS:(hq + 1) * QGS]
                    for dt in range(DT):
                        nc.vector.tensor_tensor(
                            attn_h[:, dt, qsl], attacc[:, dt, qsl], rbq,
                            ALU.mult)
                        nc.vector.tensor_tensor(
                            attn_h[:, dt, qsl], attn_h[:, dt, qsl],
                            bsb["bv"][:, dt:dt + 1].to_broadcast([P, QGS]),
                            ALU.add)

                for c, kt_n in enumerate(CHUNK_KT):
                    kn = P * kt_n
                    base = kvag[c]
                    sz = 2 * D * kn + (WSZ if c == last_c else 0)
                    for kb in range(KB):
                        off = kb * sz
                        ktb = kv.tile([P, DT, 512], F16, tag="ktb")
                        vb = kv.tile([P, 4, D], F16, tag="vb")
                        nc.sync.dma_start(
                            ktb[:, :, 0:kn],
                            base[off:off + D * kn].rearrange(
                                "(t p k) -> p t k", p=P, k=kn))
                        nc.sync.dma_start(
                            vb[:, 0:kt_n, :],
                            base[off + D * kn:off + 2 * D * kn].rearrange(
                                "(t p d) -> p t d", p=P, d=D))
                        first_blk = c == 0 and kb == 0
                        last_blk = (c == len(CHUNK_KT) - 1 and kb == KB - 1)
                        if c == last_c and kb == 2:
                            for cc in range(NCORES):
                                o = cc * sz + woff
                                nc.sync.dma_start(
                                    wsh[cc][:],
                                    base[o:o + WSZ].rearrange(
                                        "(p l i e) -> p l i e",
                                        l=3, i=DT, p=P, e=P))
                        exs = [[], []]
                        for kt in range(kt_n):
                            scs = [psc.tile([P, 512], F32, tag="psc",
                                            name=f"sc{_h}")
                                   for _h in range(2)]
                            for dt in range(DT):
                                for qp in range(2):
                                    nc.tensor.matmul(
                                        scs[qp][:],
                                        ktb[:, dt, kt * P:(kt + 1) * P],
                                        qt[:, dt, qp * 512:(qp + 1) * 512],
                                        start=(dt == 0), stop=(dt == DT - 1),
                                        skip_group_check=True)
                            for qp in range(2):
                                ex = exp_pool.tile([P, 512], F16, tag="ex",
                                                   name=f"ex{kt}_{qp}")
                                nc.scalar.activation(ex[:], scs[qp][:], AF.Exp,
                                                     scale=float(SCALE))
                                nc.vector.tensor_tensor(
                                    rs_acc[:, qp * 512:(qp + 1) * 512], ex[:],
                                    rs_acc[:, qp * 512:(qp + 1) * 512],
                                    ALU.add)
                                exs[qp].append(ex)
                        # A@V, one PSUM-bank accumulation group at a time
                        for qp in range(2):
                            for hq in range(2):
                                qsl = slice(qp * 512 + hq * QGS,
                                            qp * 512 + (hq + 1) * QGS)
                                att_ps = [pat.tile([P, 2, QGS], F32, tag="pat",
                                                   name=f"att_ps{_j}")
                                          for _j in range(4)]
                                for dt in range(DT):
                                    for kt in range(kt_n):
                                        nc.tensor.matmul(
                                            att_ps[dt // 2][:, dt % 2, :],
                                            vb[:, kt, dt * P:(dt + 1) * P],
                                            exs[qp][kt][:,
                                                        hq * QGS:(hq + 1) * QGS],
                                            start=(kt == 0),
                                            stop=(kt == kt_n - 1),
                                            skip_group_check=True)
                                for j in range(4):
                                    dsl = (slice(None), slice(2 * j, 2 * j + 2),
                                           qsl)
                                    if first_blk:
                                        nc.vector.tensor_copy(attacc[dsl],
                                                              att_ps[j][:])
                                    else:
                                        nc.vector.tensor_tensor(
                                            attacc[dsl], att_ps[j][:],
                                            attacc[dsl], ALU.add)
                                if last_blk:
                                    norm_part(qp, hq)

            # ---- MLP + final, per column-half ----
            with (
                tc.tile_pool(name="acts", bufs=2) as acts,
                tc.tile_pool(name="pml", bufs=4, space="PSUM") as pml,
            ):
                out_sb = acts.tile([1, NS], F32, tag="out_sb")
                cur = attn_h
                for li, (wname, bname) in enumerate(
                        (("w1", "b1"), ("w2", "b2"), ("w3", "b3"))):
                    nxt = acts.tile([P, DT, NS], F16, tag="y", name=f"y{li}")
                    for ft in range(DT):
                        pss = [pml.tile([P, 512], F32, tag="pml",
                                        name=f"mps{_h}")
                               for _h in range(2)]
                        for dt in range(DT):
                            for h in range(2):
                                nc.tensor.matmul(
                                    pss[h][:],
                                    wsh[ft][:, li, dt, :],
                                    cur[:, dt, h * 512:(h + 1) * 512],
                                    start=(dt == 0), stop=(dt == DT - 1),
                                    skip_group_check=True)
                        for h in range(2):
                            nc.scalar.activation(
                                nxt[:, ft, h * 512:(h + 1) * 512], pss[h][:],
                                AF.Relu, bias=bsb[bname][:, ft:ft + 1])
                    cur = nxt
                for h in range(2):
                    ps = pml.tile([1, 512], F32, tag="pfin")
                    for ft in range(DT):
                        nc.tensor.matmul(
                            ps[:], fwh[:, ft:ft + 1],
                            cur[:, ft, h * 512:(h + 1) * 512],
                            start=(ft == 0), stop=(ft == DT - 1))
                    nc.vector.tensor_copy(out_sb[0:1, h * 512:(h + 1) * 512],
                                          ps[:])
                nc.sync.dma_start(out[:], out_sb[:])
            wmlp.release()
            pacc.release()

    nc.compile()
    return nc


def _get_nc():
    if "nc" not in _CACHE:
        _CACHE["nc"] = _build()
    return _CACHE["nc"]


def _in_maps(inputs):
    x = np.ascontiguousarray(np.asarray(inputs["x"], dtype=np.float32))
    names = {"wq": "Wq", "wk": "Wk", "wv": "Wv",
             "bq": "bq", "bk": "bk", "bv": "bv", "b1": "b1",
             "b2": "b2", "b3": "b3"}
    shared = {k: np.ascontiguousarray(np.asarray(inputs[v], dtype=np.float32))
              for k, v in names.items()}
    shared["fw"] = np.ascontiguousarray(
        np.asarray(inputs["final_weight"], dtype=np.float32).reshape(D))
    w123 = [np.asarray(inputs[w], dtype=np.float32)
            for w in ("W1", "W2", "W3")]
    in_maps = []
    for c in range(NCORES):
        m = dict(shared)
        m["xs"] = np.ascontiguousarray(x[c * NS:(c + 1) * NS, :])
        for l, wfull in enumerate(w123):
            m[f"w{l + 1}s"] = np.ascontiguousarray(
                wfull[c * P:(c + 1) * P, :])
        in_maps.append(m)
    return in_maps


def kernel(**inputs):
    nc = _get_nc()
    res = bass_utils.run_bass_kernel_spmd(
        nc, _in_maps(inputs), core_ids=list(range(NCORES)))
    return np.concatenate(
        [res.results[c]["out"].reshape(NS) for c in range(NCORES)])
